# revision 21
# baseline (speedup 1.0000x reference)
"""BinaryLinear on 8 TRN2 NeuronCores.

out = sign(x) @ sign(weight).T ; x [8192, 4096] f32, weight [4096, 4096] f32.

Sharding (4x2 grid): x row-blocks of 2048 across 4 groups, weight
out_feature-blocks of 2048 across 2 groups. Core c = (mb, ob) =
(c // 2, c % 2). Each core computes out_shard [2048, 2048] =
sign(x[mb]) @ sign(w[ob]).T.

Host-side layout prep (free for the device):
  xP [16, 128, 32, 128]: xP[mt, p, kc, m] = x_shard[mt*128 + m, kc*128 + p]
     -> each m-tile's operand block is one contiguous 2 MiB DMA with
        16 KiB-contiguous reads per partition.
  wT [4096, 2048] = w_shard.T (k-major; rows are 8 KiB contiguous).

Device kernel: sign-cast both operands f32 -> fp8e4 (+-1 exact; products
+-1, fp32 PSUM accumulation of <=4096 terms is exact), keep the signed
weight shard resident in SBUF (64 KiB/partition), stream x row-tiles
through TensorE with DoubleRow matmuls (K=256 per pass, 2 MAC/cell/cyc).
"""

import numpy as np

import concourse.bass as bass
import concourse.mybir as mybir
import concourse.tile as tile
from concourse.bass_utils import run_bass_kernel_spmd
from concourse.vector_clock import ScopedClock, VectorClock

N, K, O = 8192, 4096, 4096
MB, OB = 4, 2  # shard grid
MSH, OSH = N // MB, O // OB  # 2048, 2048 per-core shard dims
KC = K // 128  # 32 k-chunks
KP = KC // 2  # 16 double-row k-pairs
MT = MSH // 128  # 16 m-tiles
NT = 512  # matmul moving free dim (psum bank)
OT = OSH // NT  # 4 o-tiles

F32 = mybir.dt.float32
FP8 = mybir.dt.float8e4
SIGN = mybir.ActivationFunctionType.Sign
DR = mybir.MatmulPerfMode.DoubleRow


def _split_drain_and_barrier(self, tick_clock, wait_clock):
    # This walrus build rejects >1 sem wait on a Drain ("Too many sync
    # wait commands"); emit one single-wait drain per active proc lane.
    gc = tick_clock.global_clock
    n = len(gc)
    for p in range(n):
        if gc[p] > 0:
            sub = VectorClock([gc[q] if q == p else 0 for q in range(n)])
            d = self.nc.sync.drain()
            wait_clock.add_sem_waits(d.ins, ScopedClock({None: sub}))
    self.nc.all_engine_barrier()
    assert self.sems is not None
    popped = self.nc._tile_sem_poison_stack.pop()
    assert popped is self._sem_poison
    self.nc.clear_and_free_semaphores(list(self.sems.allocated().values()))
    self.nc.all_engine_barrier()


tile.TileContext._drain_and_barrier = _split_drain_and_barrier

def _dedup_ldweights(nc):
    """Tile's legalize pass emits one standalone InstLdweights per
    Matmult; with o-tiles innermost, 4 consecutive Ldweights load the
    identical stationary operand. Weights persist in the PE array
    across matmuls, so drop exact-duplicate loads (preserving any sem
    waits/updates they carry via an EVSEM placeholder). DoubleRow LDW
    streams 256 columns (~213 ns, not hidden behind 107 ns matmuls), so
    this is the main PE-rate lever.
    """
    import json as _json

    import bass_rust

    def sig(ins):
        d = []
        for ap in ins.ins:
            d.append(
                (
                    getattr(ap, "memref", None),
                    getattr(ap, "offset", None),
                    str(getattr(ap, "ap", None)),
                    str(getattr(ap, "dtype", None)),
                )
            )
        return (
            str(d),
            str(getattr(ins, "perf_mode", None)),
            str(getattr(ins, "tile_position", None)),
            str(getattr(ins, "tile_size", None)),
            str(getattr(ins, "is_transpose", None)),
        )

    n_removed = 0
    for func in nc.m.functions:
        for bb in func.blocks:
            last_ldw_sig = None
            new = []
            for ins in bb.instructions:
                op = type(ins).__name__
                if ins.engine != mybir.EngineType.PE:
                    new.append(ins)
                    continue
                if op == "InstLdweights":
                    s = sig(ins)
                    if s == last_ldw_sig:
                        si = ins.sync_info
                        if si is not None and (si.on_wait or si.on_update):
                            ev = mybir.InstEventSemaphore(
                                name=ins.name + "-dedup",
                                ins=[],
                                outs=[],
                                engine=ins.engine,
                            )
                            ev.sync_info = bass_rust.SyncInfo(
                                on_wait=list(si.on_wait),
                                on_update=list(si.on_update),
                            )
                            new.append(ev)
                        n_removed += 1
                        continue
                    last_ldw_sig = s
                elif op != "InstMatmult":
                    # any other PE instruction: be conservative
                    last_ldw_sig = None
                new.append(ins)
            bb.instructions = new
    return n_removed


def _split_multi_waits(nc):
    """Walrus here allows at most ONE sem wait per instruction. Engines
    dispatch their streams in order, so waiting on k sems at one
    instruction == k single-wait EVSEMs followed by the instruction
    (for DMACopy on a DGE ring the preceding EVSEM stalls the issuing
    engine before it enqueues the descriptor — conservative, correct).
    """
    import bass_rust

    n_split = 0
    for func in nc.m.functions:
        for bb in func.blocks:
            new = []
            for ins in bb.instructions:
                si = ins.sync_info
                waits = list(si.on_wait) if si is not None else []
                if len(waits) > 1:
                    for w in waits[:-1]:
                        n_split += 1
                        ev = mybir.InstEventSemaphore(
                            name=f"I-waitsplit-{n_split}",
                            ins=[],
                            outs=[],
                            engine=ins.engine,
                        )
                        ev.sync_info = bass_rust.SyncInfo(
                            on_wait=[w], on_update=[]
                        )
                        new.append(ev)
                    ins.sync_info = bass_rust.SyncInfo(
                        on_wait=[waits[-1]], on_update=list(si.on_update)
                    )
                new.append(ins)
            bb.instructions = new
    return n_split


def build():
    nc = bass.Bass(name="bl_v8_sched")
    xP = nc.declare_dram_parameter("xP", [MT, 128, KC, 128], F32, isOutput=False)
    wT = nc.declare_dram_parameter("wT", [K, OSH], F32, isOutput=False)
    out = nc.declare_dram_parameter("out", [MSH, OSH], F32, isOutput=True)

    wT4 = wT.rearrange("(t j p) o -> p t j o", p=128, j=2)  # [128, KP, 2, OSH]

    KH = KP // 2  # 8 pairs per K-half

    with tile.TileContext(nc) as tc:
        with (
            tc.tile_pool(name="wstage", bufs=2) as wstage,
            tc.tile_pool(name="wres", bufs=KP) as wres,
            tc.tile_pool(name="xstage", bufs=1) as xstage,
            tc.tile_pool(name="xbin", bufs=6) as xbin,
            tc.tile_pool(name="part", bufs=5) as part,
            tc.tile_pool(name="psum", bufs=8, space="PSUM") as psum,
            tc.tile_pool(name="outb", bufs=4) as outb,
        ):
            wb = [None] * KP
            xbs = [None] * MT
            parts = [None] * MT

            def prep_w(t):
                wf = wstage.tile([128, 2, OSH], F32, tag="wf", name=f"wf{t}")
                nc.sync.dma_start(wf[:], wT4[:, t, :, :])
                w8 = wres.tile([128, 2, OSH], FP8, tag="wres", name=f"w8_{t}")
                nc.scalar.activation(w8[:], wf[:], SIGN)
                wb[t] = w8

            def phase_a(mt):
                # K-pairs 0..KH-1 -> f32 partial in SBUF. Runs as soon as
                # the first half of W has streamed in, long before the
                # second half arrives.
                if mt == 0:
                    prep_w(0)  # first w pair ahead of the 2 MiB x load
                xf = xstage.tile([128, KC, 128], F32, tag="xf", name=f"xf{mt}")
                nc.sync.dma_start(xf[:], xP[mt, :, :, :])
                xb = xbin.tile([128, KC, 128], FP8, tag="xb", name=f"xb{mt}")
                nc.scalar.activation(xb[:], xf[:], SIGN)
                xbs[mt] = xb
                pss = [
                    psum.tile([128, NT], F32, tag="ps", name=f"psa{mt}_{i}")
                    for i in range(OT)
                ]
                for t in range(KH):
                    if mt == 0 and t > 0:
                        prep_w(t)
                    for ot in range(OT):
                        nc.tensor.matmul(
                            pss[ot][:],
                            xb[:, 2 * t : 2 * t + 2, :],
                            wb[t][:, :, bass.ts(ot, NT)],
                            start=(t == 0),
                            stop=(t == KH - 1),
                            perf_mode=DR,
                        )
                pt = part.tile([128, OSH], F32, tag="part", name=f"part{mt}")
                for ot in range(OT):
                    nc.vector.tensor_copy(pt[:, bass.ts(ot, NT)], pss[ot][:])
                parts[mt] = pt

            def phase_b(mt):
                # K-pairs KH..KP-1, then combine with the phase-A partial.
                xb = xbs[mt]
                pss = [
                    psum.tile([128, NT], F32, tag="ps", name=f"psb{mt}_{i}")
                    for i in range(OT)
                ]
                for t in range(KH, KP):
                    if mt == 0:
                        prep_w(t)
                    for ot in range(OT):
                        nc.tensor.matmul(
                            pss[ot][:],
                            xb[:, 2 * t : 2 * t + 2, :],
                            wb[t][:, :, bass.ts(ot, NT)],
                            start=(t == KH),
                            stop=(t == KP - 1),
                            perf_mode=DR,
                        )
                pt = parts[mt]
                for ot in range(OT):
                    ob = outb.tile([128, NT], F32, tag="ob", name=f"ob{mt}_{ot}")
                    nc.vector.tensor_add(
                        ob[:], pss[ot][:], pt[:, bass.ts(ot, NT)]
                    )
                    nc.sync.dma_start(
                        out[bass.ts(mt, 128), bass.ts(ot, NT)], ob[:]
                    )

            # Early tiles keep a 3-deep phase-B backlog so phase A alone
            # consumes W-half-A while W-half-B is still streaming in; once
            # all of W is resident (mt >= 6) drain the backlog to 1 so the
            # kernel tail is a single B-phase.
            next_b = 0
            for mt in range(MT):
                phase_a(mt)
                max_backlog = 3 if mt < 6 else 1
                while mt + 1 - next_b > max_backlog:
                    phase_b(next_b)
                    next_b += 1
            while next_b < MT:
                phase_b(next_b)
                next_b += 1
    _split_multi_waits(nc)
    return nc


_CACHE = {}


def _run(in_maps, trace=False, **kwargs):
    if "nc" not in _CACHE:
        _CACHE["nc"] = build()
    try:
        return run_bass_kernel_spmd(
            _CACHE["nc"], in_maps, core_ids=list(range(8)), trace=trace, **kwargs
        )
    except Exception:
        # transient NRT_EXEC_UNIT_UNRECOVERABLE happens occasionally on
        # this fabric; the device recovers on the next attempt
        return run_bass_kernel_spmd(
            _CACHE["nc"], in_maps, core_ids=list(range(8)), trace=trace, **kwargs
        )


def _pack_x(x_shard):
    # [2048, 4096] -> [16, 128, 32, 128] with [mt, p, kc, m] indexing
    x4 = x_shard.reshape(MT, 128, KC, 128)  # [mt, m, kc, p]
    return np.ascontiguousarray(x4.transpose(0, 3, 2, 1))


def _shard(x, weight):
    in_maps = []
    for c in range(8):
        mb, ob = c // 2, c % 2
        in_maps.append(
            {
                "xP": _pack_x(x[mb * MSH : (mb + 1) * MSH, :]),
                "wT": np.ascontiguousarray(
                    weight[ob * OSH : (ob + 1) * OSH, :].T
                ),
            }
        )
    return in_maps


def _gather(results):
    out = np.empty((N, O), dtype=np.float32)
    for c in range(8):
        mb, ob = c // 2, c % 2
        out[mb * MSH : (mb + 1) * MSH, ob * OSH : (ob + 1) * OSH] = results[c][
            "out"
        ]
    return out


def kernel(x: np.ndarray, weight: np.ndarray) -> np.ndarray:
    x = np.asarray(x, dtype=np.float32)
    weight = np.asarray(weight, dtype=np.float32)
    res = _run(_shard(x, weight))
    return _gather(res.results)


# revision 22
# speedup vs baseline: 1.0436x; 1.0436x over previous
"""BinaryLinear on 8 TRN2 NeuronCores.

out = sign(x) @ sign(weight).T ; x [8192, 4096] f32, weight [4096, 4096] f32.

Sharding (4x2 grid): x row-blocks of 2048 across 4 groups, weight
out_feature-blocks of 2048 across 2 groups. Core c = (mb, ob) =
(c // 2, c % 2). Each core computes out_shard [2048, 2048] =
sign(x[mb]) @ sign(w[ob]).T.

Host-side layout prep (free for the device):
  xP [16, 128, 32, 128]: xP[mt, p, kc, m] = x_shard[mt*128 + m, kc*128 + p]
     -> each m-tile's operand block is one contiguous 2 MiB DMA with
        16 KiB-contiguous reads per partition.
  wT [4096, 2048] = w_shard.T (k-major; rows are 8 KiB contiguous).

Device kernel: sign-cast both operands f32 -> fp8e4 (+-1 exact; products
+-1, fp32 PSUM accumulation of <=4096 terms is exact), keep the signed
weight shard resident in SBUF (64 KiB/partition), stream x row-tiles
through TensorE with DoubleRow matmuls (K=256 per pass, 2 MAC/cell/cyc).
"""

import numpy as np

import concourse.bass as bass
import concourse.mybir as mybir
import concourse.tile as tile
from concourse.bass_utils import run_bass_kernel_spmd
from concourse.vector_clock import ScopedClock, VectorClock

N, K, O = 8192, 4096, 4096
MB, OB = 4, 2  # shard grid
MSH, OSH = N // MB, O // OB  # 2048, 2048 per-core shard dims
KC = K // 128  # 32 k-chunks
KP = KC // 2  # 16 double-row k-pairs
MT = MSH // 128  # 16 m-tiles
NT = 512  # matmul moving free dim (psum bank)
OT = OSH // NT  # 4 o-tiles

F32 = mybir.dt.float32
FP8 = mybir.dt.float8e4
SIGN = mybir.ActivationFunctionType.Sign
DR = mybir.MatmulPerfMode.DoubleRow


def _split_drain_and_barrier(self, tick_clock, wait_clock):
    # This walrus build rejects >1 sem wait on a Drain ("Too many sync
    # wait commands"); emit one single-wait drain per active proc lane.
    gc = tick_clock.global_clock
    n = len(gc)
    for p in range(n):
        if gc[p] > 0:
            sub = VectorClock([gc[q] if q == p else 0 for q in range(n)])
            d = self.nc.sync.drain()
            wait_clock.add_sem_waits(d.ins, ScopedClock({None: sub}))
    self.nc.all_engine_barrier()
    assert self.sems is not None
    popped = self.nc._tile_sem_poison_stack.pop()
    assert popped is self._sem_poison
    self.nc.clear_and_free_semaphores(list(self.sems.allocated().values()))
    self.nc.all_engine_barrier()


tile.TileContext._drain_and_barrier = _split_drain_and_barrier

def _dedup_ldweights(nc):
    """Tile's legalize pass emits one standalone InstLdweights per
    Matmult; with o-tiles innermost, 4 consecutive Ldweights load the
    identical stationary operand. Weights persist in the PE array
    across matmuls, so drop exact-duplicate loads (preserving any sem
    waits/updates they carry via an EVSEM placeholder). DoubleRow LDW
    streams 256 columns (~213 ns, not hidden behind 107 ns matmuls), so
    this is the main PE-rate lever.
    """
    import json as _json

    import bass_rust

    def sig(ins):
        d = []
        for ap in ins.ins:
            d.append(
                (
                    getattr(ap, "memref", None),
                    getattr(ap, "offset", None),
                    str(getattr(ap, "ap", None)),
                    str(getattr(ap, "dtype", None)),
                )
            )
        return (
            str(d),
            str(getattr(ins, "perf_mode", None)),
            str(getattr(ins, "tile_position", None)),
            str(getattr(ins, "tile_size", None)),
            str(getattr(ins, "is_transpose", None)),
        )

    n_removed = 0
    for func in nc.m.functions:
        for bb in func.blocks:
            last_ldw_sig = None
            new = []
            for ins in bb.instructions:
                op = type(ins).__name__
                if ins.engine != mybir.EngineType.PE:
                    new.append(ins)
                    continue
                if op == "InstLdweights":
                    s = sig(ins)
                    if s == last_ldw_sig:
                        si = ins.sync_info
                        if si is not None and (si.on_wait or si.on_update):
                            ev = mybir.InstEventSemaphore(
                                name=ins.name + "-dedup",
                                ins=[],
                                outs=[],
                                engine=ins.engine,
                            )
                            ev.sync_info = bass_rust.SyncInfo(
                                on_wait=list(si.on_wait),
                                on_update=list(si.on_update),
                            )
                            new.append(ev)
                        n_removed += 1
                        continue
                    last_ldw_sig = s
                elif op != "InstMatmult":
                    # any other PE instruction: be conservative
                    last_ldw_sig = None
                new.append(ins)
            bb.instructions = new
    return n_removed


def _split_multi_waits(nc):
    """Walrus here allows at most ONE sem wait per instruction. Engines
    dispatch their streams in order, so waiting on k sems at one
    instruction == k single-wait EVSEMs followed by the instruction
    (for DMACopy on a DGE ring the preceding EVSEM stalls the issuing
    engine before it enqueues the descriptor — conservative, correct).
    """
    import bass_rust

    n_split = 0
    for func in nc.m.functions:
        for bb in func.blocks:
            new = []
            for ins in bb.instructions:
                si = ins.sync_info
                waits = list(si.on_wait) if si is not None else []
                if len(waits) > 1:
                    for w in waits[:-1]:
                        n_split += 1
                        ev = mybir.InstEventSemaphore(
                            name=f"I-waitsplit-{n_split}",
                            ins=[],
                            outs=[],
                            engine=ins.engine,
                        )
                        ev.sync_info = bass_rust.SyncInfo(
                            on_wait=[w], on_update=[]
                        )
                        new.append(ev)
                    ins.sync_info = bass_rust.SyncInfo(
                        on_wait=[waits[-1]], on_update=list(si.on_update)
                    )
                new.append(ins)
            bb.instructions = new
    return n_split


def build():
    nc = bass.Bass(name="bl_v9_deep")
    xP = nc.declare_dram_parameter("xP", [MT, 128, KC, 128], F32, isOutput=False)
    wT = nc.declare_dram_parameter("wT", [K, OSH], F32, isOutput=False)
    out = nc.declare_dram_parameter("out", [MSH, OSH], F32, isOutput=True)

    wT4 = wT.rearrange("(t j p) o -> p t j o", p=128, j=2)  # [128, KP, 2, OSH]

    KH = KP // 2  # 8 pairs per K-half

    with tile.TileContext(nc) as tc:
        with (
            tc.tile_pool(name="wstage", bufs=2) as wstage,
            tc.tile_pool(name="wres", bufs=KP) as wres,
            tc.tile_pool(name="xstage", bufs=1) as xstage,
            tc.tile_pool(name="xbin", bufs=8) as xbin,
            tc.tile_pool(name="part", bufs=7) as part,
            tc.tile_pool(name="psum", bufs=8, space="PSUM") as psum,
            tc.tile_pool(name="outb", bufs=4) as outb,
        ):
            wb = [None] * KP
            xbs = [None] * MT
            parts = [None] * MT

            def prep_w(t):
                wf = wstage.tile([128, 2, OSH], F32, tag="wf", name=f"wf{t}")
                nc.sync.dma_start(wf[:], wT4[:, t, :, :])
                w8 = wres.tile([128, 2, OSH], FP8, tag="wres", name=f"w8_{t}")
                nc.scalar.activation(w8[:], wf[:], SIGN)
                wb[t] = w8

            def phase_a(mt):
                # K-pairs 0..KH-1 -> f32 partial in SBUF. Runs as soon as
                # the first half of W has streamed in, long before the
                # second half arrives.
                if mt == 0:
                    prep_w(0)  # first w pair ahead of the 2 MiB x load
                xf = xstage.tile([128, KC, 128], F32, tag="xf", name=f"xf{mt}")
                nc.sync.dma_start(xf[:], xP[mt, :, :, :])
                xb = xbin.tile([128, KC, 128], FP8, tag="xb", name=f"xb{mt}")
                nc.scalar.activation(xb[:], xf[:], SIGN)
                xbs[mt] = xb
                pss = [
                    psum.tile([128, NT], F32, tag="ps", name=f"psa{mt}_{i}")
                    for i in range(OT)
                ]
                for t in range(KH):
                    if mt == 0 and t > 0:
                        prep_w(t)
                    for ot in range(OT):
                        nc.tensor.matmul(
                            pss[ot][:],
                            xb[:, 2 * t : 2 * t + 2, :],
                            wb[t][:, :, bass.ts(ot, NT)],
                            start=(t == 0),
                            stop=(t == KH - 1),
                            perf_mode=DR,
                        )
                pt = part.tile([128, OSH], mybir.dt.float16, tag="part", name=f"part{mt}")
                for ot in range(OT):
                    nc.vector.tensor_copy(pt[:, bass.ts(ot, NT)], pss[ot][:])
                parts[mt] = pt

            def phase_b(mt):
                # K-pairs KH..KP-1, then combine with the phase-A partial.
                xb = xbs[mt]
                pss = [
                    psum.tile([128, NT], F32, tag="ps", name=f"psb{mt}_{i}")
                    for i in range(OT)
                ]
                for t in range(KH, KP):
                    if mt == 0:
                        prep_w(t)
                    for ot in range(OT):
                        nc.tensor.matmul(
                            pss[ot][:],
                            xb[:, 2 * t : 2 * t + 2, :],
                            wb[t][:, :, bass.ts(ot, NT)],
                            start=(t == KH),
                            stop=(t == KP - 1),
                            perf_mode=DR,
                        )
                pt = parts[mt]
                for ot in range(OT):
                    ob = outb.tile([128, NT], F32, tag="ob", name=f"ob{mt}_{ot}")
                    nc.vector.tensor_add(
                        ob[:], pss[ot][:], pt[:, bass.ts(ot, NT)]
                    )
                    nc.sync.dma_start(
                        out[bass.ts(mt, 128), bass.ts(ot, NT)], ob[:]
                    )

            # Early tiles keep a 3-deep phase-B backlog so phase A alone
            # consumes W-half-A while W-half-B is still streaming in; once
            # all of W is resident (mt >= 6) drain the backlog to 1 so the
            # kernel tail is a single B-phase.
            next_b = 0
            for mt in range(MT):
                phase_a(mt)
                max_backlog = 5 if mt < 6 else 1
                while mt + 1 - next_b > max_backlog:
                    phase_b(next_b)
                    next_b += 1
            while next_b < MT:
                phase_b(next_b)
                next_b += 1
    _split_multi_waits(nc)
    return nc


_CACHE = {}


def _run(in_maps, trace=False, **kwargs):
    if "nc" not in _CACHE:
        _CACHE["nc"] = build()
    try:
        return run_bass_kernel_spmd(
            _CACHE["nc"], in_maps, core_ids=list(range(8)), trace=trace, **kwargs
        )
    except Exception:
        # transient NRT_EXEC_UNIT_UNRECOVERABLE happens occasionally on
        # this fabric; the device recovers on the next attempt
        return run_bass_kernel_spmd(
            _CACHE["nc"], in_maps, core_ids=list(range(8)), trace=trace, **kwargs
        )


def _pack_x(x_shard):
    # [2048, 4096] -> [16, 128, 32, 128] with [mt, p, kc, m] indexing
    x4 = x_shard.reshape(MT, 128, KC, 128)  # [mt, m, kc, p]
    return np.ascontiguousarray(x4.transpose(0, 3, 2, 1))


def _shard(x, weight):
    in_maps = []
    for c in range(8):
        mb, ob = c // 2, c % 2
        in_maps.append(
            {
                "xP": _pack_x(x[mb * MSH : (mb + 1) * MSH, :]),
                "wT": np.ascontiguousarray(
                    weight[ob * OSH : (ob + 1) * OSH, :].T
                ),
            }
        )
    return in_maps


def _gather(results):
    out = np.empty((N, O), dtype=np.float32)
    for c in range(8):
        mb, ob = c // 2, c % 2
        out[mb * MSH : (mb + 1) * MSH, ob * OSH : (ob + 1) * OSH] = results[c][
            "out"
        ]
    return out


def kernel(x: np.ndarray, weight: np.ndarray) -> np.ndarray:
    x = np.asarray(x, dtype=np.float32)
    weight = np.asarray(weight, dtype=np.float32)
    res = _run(_shard(x, weight))
    return _gather(res.results)


# revision 24
# speedup vs baseline: 1.0671x; 1.0225x over previous
"""BinaryLinear on 8 TRN2 NeuronCores.

out = sign(x) @ sign(weight).T ; x [8192, 4096] f32, weight [4096, 4096] f32.

Sharding (4x2 grid): x row-blocks of 2048 across 4 groups, weight
out_feature-blocks of 2048 across 2 groups. Core c = (mb, ob) =
(c // 2, c % 2). Each core computes out_shard [2048, 2048] =
sign(x[mb]) @ sign(w[ob]).T.

Host-side layout prep (free for the device):
  xP [16, 128, 32, 128]: xP[mt, p, kc, m] = x_shard[mt*128 + m, kc*128 + p]
     -> each m-tile's operand block is one contiguous 2 MiB DMA with
        16 KiB-contiguous reads per partition.
  wT [4096, 2048] = w_shard.T (k-major; rows are 8 KiB contiguous).

Device kernel: sign-cast both operands f32 -> fp8e4 (+-1 exact; products
+-1, fp32 PSUM accumulation of <=4096 terms is exact), keep the signed
weight shard resident in SBUF (64 KiB/partition), stream x row-tiles
through TensorE with DoubleRow matmuls (K=256 per pass, 2 MAC/cell/cyc).
"""

import numpy as np

import concourse.bass as bass
import concourse.mybir as mybir
import concourse.tile as tile
from concourse.bass_utils import run_bass_kernel_spmd
from concourse.vector_clock import ScopedClock, VectorClock

N, K, O = 8192, 4096, 4096
MB, OB = 4, 2  # shard grid
MSH, OSH = N // MB, O // OB  # 2048, 2048 per-core shard dims
KC = K // 128  # 32 k-chunks
KP = KC // 2  # 16 double-row k-pairs
MT = MSH // 128  # 16 m-tiles
NT = 512  # matmul moving free dim (psum bank)
OT = OSH // NT  # 4 o-tiles

F32 = mybir.dt.float32
FP8 = mybir.dt.float8e4
SIGN = mybir.ActivationFunctionType.Sign
DR = mybir.MatmulPerfMode.DoubleRow


def _split_drain_and_barrier(self, tick_clock, wait_clock):
    # This walrus build rejects >1 sem wait on a Drain ("Too many sync
    # wait commands"); emit one single-wait drain per active proc lane.
    gc = tick_clock.global_clock
    n = len(gc)
    for p in range(n):
        if gc[p] > 0:
            sub = VectorClock([gc[q] if q == p else 0 for q in range(n)])
            d = self.nc.sync.drain()
            wait_clock.add_sem_waits(d.ins, ScopedClock({None: sub}))
    self.nc.all_engine_barrier()
    assert self.sems is not None
    popped = self.nc._tile_sem_poison_stack.pop()
    assert popped is self._sem_poison
    self.nc.clear_and_free_semaphores(list(self.sems.allocated().values()))
    self.nc.all_engine_barrier()


tile.TileContext._drain_and_barrier = _split_drain_and_barrier

def _dedup_ldweights(nc):
    """Tile's legalize pass emits one standalone InstLdweights per
    Matmult; with o-tiles innermost, 4 consecutive Ldweights load the
    identical stationary operand. Weights persist in the PE array
    across matmuls, so drop exact-duplicate loads (preserving any sem
    waits/updates they carry via an EVSEM placeholder). DoubleRow LDW
    streams 256 columns (~213 ns, not hidden behind 107 ns matmuls), so
    this is the main PE-rate lever.
    """
    import json as _json

    import bass_rust

    def sig(ins):
        d = []
        for ap in ins.ins:
            d.append(
                (
                    getattr(ap, "memref", None),
                    getattr(ap, "offset", None),
                    str(getattr(ap, "ap", None)),
                    str(getattr(ap, "dtype", None)),
                )
            )
        return (
            str(d),
            str(getattr(ins, "perf_mode", None)),
            str(getattr(ins, "tile_position", None)),
            str(getattr(ins, "tile_size", None)),
            str(getattr(ins, "is_transpose", None)),
        )

    n_removed = 0
    for func in nc.m.functions:
        for bb in func.blocks:
            last_ldw_sig = None
            new = []
            for ins in bb.instructions:
                op = type(ins).__name__
                if ins.engine != mybir.EngineType.PE:
                    new.append(ins)
                    continue
                if op == "InstLdweights":
                    s = sig(ins)
                    if s == last_ldw_sig:
                        si = ins.sync_info
                        if si is not None and (si.on_wait or si.on_update):
                            ev = mybir.InstEventSemaphore(
                                name=ins.name + "-dedup",
                                ins=[],
                                outs=[],
                                engine=ins.engine,
                            )
                            ev.sync_info = bass_rust.SyncInfo(
                                on_wait=list(si.on_wait),
                                on_update=list(si.on_update),
                            )
                            new.append(ev)
                        n_removed += 1
                        continue
                    last_ldw_sig = s
                elif op != "InstMatmult":
                    # any other PE instruction: be conservative
                    last_ldw_sig = None
                new.append(ins)
            bb.instructions = new
    return n_removed


def _split_multi_waits(nc):
    """Walrus here allows at most ONE sem wait per instruction. Engines
    dispatch their streams in order, so waiting on k sems at one
    instruction == k single-wait EVSEMs followed by the instruction
    (for DMACopy on a DGE ring the preceding EVSEM stalls the issuing
    engine before it enqueues the descriptor — conservative, correct).
    """
    import bass_rust

    n_split = 0
    for func in nc.m.functions:
        for bb in func.blocks:
            new = []
            for ins in bb.instructions:
                si = ins.sync_info
                waits = list(si.on_wait) if si is not None else []
                if len(waits) > 1:
                    for w in waits[:-1]:
                        n_split += 1
                        ev = mybir.InstEventSemaphore(
                            name=f"I-waitsplit-{n_split}",
                            ins=[],
                            outs=[],
                            engine=ins.engine,
                        )
                        ev.sync_info = bass_rust.SyncInfo(
                            on_wait=[w], on_update=[]
                        )
                        new.append(ev)
                    ins.sync_info = bass_rust.SyncInfo(
                        on_wait=[waits[-1]], on_update=list(si.on_update)
                    )
                new.append(ins)
            bb.instructions = new
    return n_split


def build():
    nc = bass.Bass(name="bl_v10_dmasplit")
    xP = nc.declare_dram_parameter("xP", [MT, 128, KC, 128], F32, isOutput=False)
    wT = nc.declare_dram_parameter("wT", [K, OSH], F32, isOutput=False)
    out = nc.declare_dram_parameter("out", [MSH, OSH], F32, isOutput=True)

    wT4 = wT.rearrange("(t j p) o -> p t j o", p=128, j=2)  # [128, KP, 2, OSH]

    KH = KP // 2  # 8 pairs per K-half

    with tile.TileContext(nc) as tc:
        with (
            tc.tile_pool(name="wstage", bufs=2) as wstage,
            tc.tile_pool(name="wres", bufs=KP) as wres,
            tc.tile_pool(name="xstage", bufs=1) as xstage,
            tc.tile_pool(name="xbin", bufs=6) as xbin,
            tc.tile_pool(name="part", bufs=5) as part,
            tc.tile_pool(name="psum", bufs=8, space="PSUM") as psum,
            tc.tile_pool(name="outb", bufs=4) as outb,
        ):
            wb = [None] * KP
            xbs = [None] * MT
            parts = [None] * MT

            def prep_w(t):
                wf = wstage.tile([128, 2, OSH], F32, tag="wf", name=f"wf{t}")
                nc.gpsimd.dma_start(wf[:], wT4[:, t, :, :])
                w8 = wres.tile([128, 2, OSH], FP8, tag="wres", name=f"w8_{t}")
                nc.scalar.activation(w8[:], wf[:], SIGN)
                wb[t] = w8

            def phase_a(mt):
                # K-pairs 0..KH-1 -> f32 partial in SBUF. Runs as soon as
                # the first half of W has streamed in, long before the
                # second half arrives.
                if mt == 0:
                    prep_w(0)  # first w pair ahead of the 2 MiB x load
                xf = xstage.tile([128, KC, 128], F32, tag="xf", name=f"xf{mt}")
                nc.sync.dma_start(xf[:], xP[mt, :, :, :])
                xb = xbin.tile([128, KC, 128], FP8, tag="xb", name=f"xb{mt}")
                nc.scalar.activation(xb[:], xf[:], SIGN)
                xbs[mt] = xb
                pss = [
                    psum.tile([128, NT], F32, tag="ps", name=f"psa{mt}_{i}")
                    for i in range(OT)
                ]
                for t in range(KH):
                    if mt == 0 and t > 0:
                        prep_w(t)
                    for ot in range(OT):
                        nc.tensor.matmul(
                            pss[ot][:],
                            xb[:, 2 * t : 2 * t + 2, :],
                            wb[t][:, :, bass.ts(ot, NT)],
                            start=(t == 0),
                            stop=(t == KH - 1),
                            perf_mode=DR,
                        )
                pt = part.tile([128, OSH], F32, tag="part", name=f"part{mt}")
                for ot in range(OT):
                    nc.vector.tensor_copy(pt[:, bass.ts(ot, NT)], pss[ot][:])
                parts[mt] = pt

            def phase_b(mt):
                # K-pairs KH..KP-1, then combine with the phase-A partial.
                xb = xbs[mt]
                pss = [
                    psum.tile([128, NT], F32, tag="ps", name=f"psb{mt}_{i}")
                    for i in range(OT)
                ]
                for t in range(KH, KP):
                    if mt == 0:
                        prep_w(t)
                    for ot in range(OT):
                        nc.tensor.matmul(
                            pss[ot][:],
                            xb[:, 2 * t : 2 * t + 2, :],
                            wb[t][:, :, bass.ts(ot, NT)],
                            start=(t == KH),
                            stop=(t == KP - 1),
                            perf_mode=DR,
                        )
                pt = parts[mt]
                for ot in range(OT):
                    ob = outb.tile([128, NT], F32, tag="ob", name=f"ob{mt}_{ot}")
                    nc.vector.tensor_add(
                        ob[:], pss[ot][:], pt[:, bass.ts(ot, NT)]
                    )
                    nc.gpsimd.dma_start(
                        out[bass.ts(mt, 128), bass.ts(ot, NT)], ob[:]
                    )

            # Early tiles keep a 3-deep phase-B backlog so phase A alone
            # consumes W-half-A while W-half-B is still streaming in; once
            # all of W is resident (mt >= 6) drain the backlog to 1 so the
            # kernel tail is a single B-phase.
            next_b = 0
            for mt in range(MT):
                phase_a(mt)
                max_backlog = 3 if mt < 6 else 1
                while mt + 1 - next_b > max_backlog:
                    phase_b(next_b)
                    next_b += 1
            while next_b < MT:
                phase_b(next_b)
                next_b += 1
    _split_multi_waits(nc)
    return nc


_CACHE = {}


def _run(in_maps, trace=False, **kwargs):
    if "nc" not in _CACHE:
        _CACHE["nc"] = build()
    try:
        return run_bass_kernel_spmd(
            _CACHE["nc"], in_maps, core_ids=list(range(8)), trace=trace, **kwargs
        )
    except Exception:
        # transient NRT_EXEC_UNIT_UNRECOVERABLE happens occasionally on
        # this fabric; the device recovers on the next attempt
        return run_bass_kernel_spmd(
            _CACHE["nc"], in_maps, core_ids=list(range(8)), trace=trace, **kwargs
        )


def _pack_x(x_shard):
    # [2048, 4096] -> [16, 128, 32, 128] with [mt, p, kc, m] indexing
    x4 = x_shard.reshape(MT, 128, KC, 128)  # [mt, m, kc, p]
    return np.ascontiguousarray(x4.transpose(0, 3, 2, 1))


def _shard(x, weight):
    in_maps = []
    for c in range(8):
        mb, ob = c // 2, c % 2
        in_maps.append(
            {
                "xP": _pack_x(x[mb * MSH : (mb + 1) * MSH, :]),
                "wT": np.ascontiguousarray(
                    weight[ob * OSH : (ob + 1) * OSH, :].T
                ),
            }
        )
    return in_maps


def _gather(results):
    out = np.empty((N, O), dtype=np.float32)
    for c in range(8):
        mb, ob = c // 2, c % 2
        out[mb * MSH : (mb + 1) * MSH, ob * OSH : (ob + 1) * OSH] = results[c][
            "out"
        ]
    return out


def kernel(x: np.ndarray, weight: np.ndarray) -> np.ndarray:
    x = np.asarray(x, dtype=np.float32)
    weight = np.asarray(weight, dtype=np.float32)
    res = _run(_shard(x, weight))
    return _gather(res.results)
